# revision 1
# baseline (speedup 1.0000x reference)
"""SpGAT_Conv Trainium2 kernel: 8-core SPMD spectral GNN conv.

Math (reference):
    a = softmax(alpha)
    pre = x @ W                                   [N, D]
    out_low  = s0 @ (a0 * (s1 @ pre))             [N, D]
    out_high = s2 @ (a1 * (s3 @ pre))             [N, D]
    out = relu(max(out_low, out_high) + bias)

Sharding: row-shard the node dim N across 8 cores.  Let S = concat(s1, s3)
(rows 0..N-1).  Core c owns rows [1024c, 1024c+1024):
    phase 1: pre = x @ W computed fully on every core (replicated; cheaper
             than the gather it replaces and fills the collective-init window)
    phase 2: t_c = S_c @ pre in two 4-strip sweeps; each sweep's rows are
             AllGathered (4 sub-collectives total) while the next sweep /
             phase 3 computes -> t = concat(t1, t3)
    phase 3: out_c = relu(max(a0 * s0_c @ t1, a1 * s2_c @ t3) + bias), in two
             4-strip halves so the first half's epilogue+stores overlap the
             second half's matmuls.

All big operands are pre-transposed host-side during sharding so the PE's
contraction dim lands on SBUF partitions with plain contiguous DMAs
(fp32 has no DMA-transpose path on trn2).  Compute dtype is bf16
(host-cast; full PE rate) with fp32 PSUM accumulation; set
SPGAT_COMPUTE=f32r for the float32r variant.

A dependency-free tiny AllGather is issued at kernel start to absorb the
first-collective init + inter-core launch skew.
"""

import os

import numpy as np

N_CORES = 8
N = 8192
K = 2048
NK = N - K          # 6144
D = 512
ROWS = N // N_CORES  # 1024 rows per core
P = 128
RCH = ROWS // P      # 8  (row chunks per core / output strips)
NCH = N // P         # 64 (contraction chunks over full N)
KCH = K // P         # 16 (low-band chunks; high band = NCH - KCH = 48)
NSUB = 4             # sub-AllGathers for t
SUBR = ROWS // NSUB  # 256 rows per rank per sub-AG

COMPUTE = os.environ.get("SPGAT_COMPUTE", "bf16")  # "bf16" | "f32r"
DEBUG = os.environ.get("SPGAT_DEBUG", "0") == "1"

_CACHE = {}

# t-chunk arrival order: sub-AG g delivers, for every rank c, t rows
# [1024c + 256g, 1024c + 256(g+1)) = global chunks 8c + 2g + {0,1}.
# Gathers are triggered in order 0, 1, 3, 2 (sub 3's rows are staged first
# after sweep 2), so consume in that order too.
GORDER = [0, 1, 3, 2]
ARRIVAL = [
    (8 * c + 2 * g + u, g, c, u)
    for g in GORDER
    for c in range(N_CORES)
    for u in range(2)
]


def _build_nc(compute):
    import concourse.mybir as mybir
    import concourse.tile as tile
    from concourse import bacc

    f32 = mybir.dt.float32
    bf16 = mybir.dt.bfloat16
    f32r = mybir.dt.float32r
    cdt = bf16 if compute == "bf16" else f32   # storage dtype of matmul operands

    def mmcast(ap):
        return ap.bitcast(f32r) if compute == "f32r" else ap

    nc = bacc.Bacc(
        "TRN2", target_bir_lowering=False, debug=False, num_devices=N_CORES
    )

    xt = nc.dram_tensor("xt", [D, N], cdt, kind="ExternalInput").ap()
    w = nc.dram_tensor("w", [D, D], cdt, kind="ExternalInput").ap()
    alpha = nc.dram_tensor("alpha", [2], f32, kind="ExternalInput").ap()
    bias = nc.dram_tensor("bias", [D], f32, kind="ExternalInput").ap()
    st = nc.dram_tensor("st", [N, ROWS], cdt, kind="ExternalInput").ap()
    s0t = nc.dram_tensor("s0t", [K, ROWS], cdt, kind="ExternalInput").ap()
    s2t = nc.dram_tensor("s2t", [NK, ROWS], cdt, kind="ExternalInput").ap()
    out = nc.dram_tensor("out", [ROWS, D], f32, kind="ExternalOutput").ap()
    if DEBUG:
        pre_dump = nc.dram_tensor("pre_dump", [N, D], cdt, kind="ExternalOutput").ap()
        t_dump = nc.dram_tensor("t_dump", [N, D], cdt, kind="ExternalOutput").ap()

    groups = [list(range(N_CORES))]

    with tile.TileContext(nc) as tc:
        with (
            tc.tile_pool(name="const", bufs=1) as const,
            tc.tile_pool(name="bigA", bufs=1) as bigA,
            tc.tile_pool(name="bigB", bufs=1) as bigB,
            tc.tile_pool(name="strips", bufs=10) as strips,
            tc.tile_pool(name="stage", bufs=6) as stage,
            tc.tile_pool(name="stash", bufs=1) as stashp,
            tc.tile_pool(name="ps", bufs=8, space="PSUM") as ps,
            tc.tile_pool(name="dram", bufs=1, space="DRAM") as dram,
        ):
            # ---- collective warm-up: absorb first-collective init + launch
            # skew.  No input deps (values are irrelevant) so the trigger is
            # the very first gpsimd instruction.
            warm_in = dram.tile([8, 8], f32, name="warm_in")
            warm_out = dram.tile([64, 8], f32, name="warm_out", addr_space="Shared")
            nc.gpsimd.collective_compute(
                "AllGather",
                mybir.AluOpType.bypass,
                replica_groups=groups,
                ins=[warm_in.opt()],
                outs=[warm_out.opt()],
            )

            # ---- setup: softmax(alpha), broadcast a and bias to 128 partitions
            asb = const.tile([1, 2], f32, name="asb")
            nc.sync.dma_start(asb[:], alpha[None, :])
            bsb = const.tile([1, D], f32, name="bsb")
            nc.sync.dma_start(bsb[:], bias[None, :])

            amax = const.tile([1, 1], f32, name="amax")
            nc.vector.tensor_tensor(
                amax[:], asb[:, 0:1], asb[:, 1:2], mybir.AluOpType.max
            )
            ash = const.tile([1, 2], f32, name="ash")
            nc.vector.tensor_scalar(
                ash[:], asb[:], amax[:, 0:1], None, mybir.AluOpType.subtract
            )
            aexp = const.tile([1, 2], f32, name="aexp")
            nc.scalar.activation(aexp[:], ash[:], mybir.ActivationFunctionType.Exp)
            asum = const.tile([1, 1], f32, name="asum")
            nc.vector.tensor_tensor(
                asum[:], aexp[:, 0:1], aexp[:, 1:2], mybir.AluOpType.add
            )
            arec = const.tile([1, 1], f32, name="arec")
            nc.vector.reciprocal(arec[:], asum[:])
            afin = const.tile([1, 2], f32, name="afin")
            nc.vector.tensor_scalar(
                afin[:], aexp[:], arec[:, 0:1], None, mybir.AluOpType.mult
            )

            ones = const.tile([1, P], f32, name="ones")
            nc.vector.memset(ones[:], 1.0)
            ps_a = ps.tile([P, 2], f32, name="ps_a", tag="acc")
            nc.tensor.matmul(ps_a[:], ones[:], afin[:], start=True, stop=True)
            a128 = const.tile([P, 2], f32, name="a128")
            nc.vector.tensor_copy(a128[:], ps_a[:])
            ps_b = ps.tile([P, D], f32, name="ps_b", tag="acc")
            nc.tensor.matmul(ps_b[:], ones[:], bsb[:], start=True, stop=True)
            bias128 = const.tile([P, D], f32, name="bias128")
            nc.vector.tensor_copy(bias128[:], ps_b[:])

            # ---- phase 1: pre = x @ W, computed fully on every core,
            # landing directly in SBUF (no gather, no DRAM bounce)
            w_sb = const.tile([P, D // P, D], cdt, name="w_sb")
            nc.sync.dma_start(w_sb[:], w.rearrange("(c p) d -> p c d", p=P))
            xt_sb = bigB.tile([P, D // P, N], cdt, name="xt_sb", tag="bigB")
            XB = 16  # chunked loads so phase-1 can start early
            xt_v = xt.rearrange("(c p) (b n) -> b p c n", p=P, b=XB)
            xw = N // XB
            for b in range(XB):
                if b == 0:  # split so the first phase-1 matmul starts sooner
                    nc.sync.dma_start(xt_sb[:, 0:2, 0:xw], xt_v[0][:, 0:2, :])
                    nc.sync.dma_start(xt_sb[:, 2:4, 0:xw], xt_v[0][:, 2:4, :])
                else:
                    nc.sync.dma_start(xt_sb[:, :, xw * b : xw * (b + 1)], xt_v[b])

            pre_sb = bigA.tile([P, NCH, D], cdt, name="pre_sb", tag="bigA")
            for j in range(NCH):
                acc = ps.tile([P, D], f32, name=f"acc1_{j}", tag="acc")
                for dc in range(D // P):
                    nc.tensor.matmul(
                        acc[:],
                        mmcast(xt_sb[:, dc, P * j : P * (j + 1)]),
                        mmcast(w_sb[:, dc, :]),
                        start=(dc == 0),
                        stop=(dc == D // P - 1),
                    )
                if j % 2 == 0:  # alternate engines: faster PSUM bank release
                    nc.vector.tensor_copy(pre_sb[:, j, :], acc[:])
                else:
                    nc.scalar.copy(pre_sb[:, j, :], acc[:])

            if DEBUG:
                for j in range(NCH):
                    nc.sync.dma_start(
                        pre_dump[P * j : P * (j + 1), :], pre_sb[:, j, :]
                    )

            # ---- phase 2: t_c = S_c @ pre, two sweeps of 4 output strips;
            # each sweep's t rows are AllGathered in two sub-collectives
            t_in = dram.tile([ROWS, D], cdt, name="t_in")
            t_outs = [
                dram.tile([SUBR * N_CORES, D], cdt, name=f"t_out{g}",
                          addr_space="Shared")
                for g in range(NSUB)
            ]

            def t_subag(g):
                nc.gpsimd.collective_compute(
                    "AllGather",
                    mybir.AluOpType.bypass,
                    replica_groups=groups,
                    ins=[t_in[SUBR * g : SUBR * (g + 1), :].opt()],
                    outs=[t_outs[g].opt()],
                )

            SW = RCH // 2  # 4 strips per sweep
            for sw in range(2):
                col0 = P * SW * sw
                accs2 = [
                    ps.tile([P, D], f32, name=f"acc2_{sw}_{kt}", tag="acc")
                    for kt in range(SW)
                ]
                for j in range(NCH):
                    strip = strips.tile([P, ROWS], cdt, name=f"s{sw}_{j}",
                                        tag="strip")
                    sl = strip[:, : P * SW]
                    nc.sync.dma_start(
                        sl, st[P * j : P * (j + 1), col0 : col0 + P * SW]
                    )
                    for kt in range(SW):
                        nc.tensor.matmul(
                            accs2[kt][:],
                            mmcast(sl[:, P * kt : P * (kt + 1)]),
                            mmcast(pre_sb[:, j, :]),
                            start=(j == 0),
                            stop=(j == NCH - 1),
                        )
                # sweep 0: stage kt 0,1 -> sub-AG 0, then kt 2,3 -> sub-AG 1
                # sweep 1: stage kt 3,2 (rows 768..1023) -> sub-AG 3 first,
                #          then kt 1,0 -> sub-AG 2 (consumed last in GORDER)
                kt_order = [0, 1, 2, 3] if sw == 0 else [3, 2, 1, 0]
                for i, kt in enumerate(kt_order):
                    row0 = P * (SW * sw + kt)
                    tst = stage.tile([P, D], cdt, name=f"t_st_{sw}_{kt}", tag="st")
                    if i % 2 == 0:
                        nc.vector.tensor_copy(tst[:], accs2[kt][:])
                    else:
                        nc.scalar.copy(tst[:], accs2[kt][:])
                    nc.sync.dma_start(t_in[row0 : row0 + P, :], tst[:])
                    if i == 1:
                        t_subag(2 * sw if sw == 0 else 3)
                    elif i == 3:
                        t_subag(2 * sw + 1 if sw == 0 else 2)

            # ---- phase 3: out_c = relu(max(a0*s0_c@t1, a1*s2_c@t3) + bias)
            t_sb = bigB.tile([P, NCH, D], cdt, name="t_sb", tag="bigB")
            for j, g, c, u in ARRIVAL:
                nc.sync.dma_start(
                    t_sb[:, j, :],
                    t_outs[g][SUBR * c + P * u : SUBR * c + P * (u + 1), :],
                )
            if DEBUG:
                for j, g, c, u in ARRIVAL:
                    nc.sync.dma_start(
                        t_dump[P * j : P * (j + 1), :], t_sb[:, j, :]
                    )

            # high band first across all 8 out strips (full-width strips, one
            # load per chunk; its 101us of matmuls covers the t-gather
            # pipeline), stash a1*high to SBUF, then the low band reuses the
            # same 8 PSUM banks with everything already gathered.
            HI_CHUNKS = [e for e in ARRIVAL if e[0] >= KCH]
            LO_CHUNKS = [e for e in ARRIVAL if e[0] < KCH]
            accs3 = [
                ps.tile([P, D], f32, name=f"acc3_{nt}", tag="acc")
                for nt in range(RCH)
            ]
            stash = [
                stashp.tile([P, D], f32, name=f"hst_{nt}", tag=f"hst{nt}")
                for nt in range(RCH)
            ]
            for idx, (j, g, c, u) in enumerate(HI_CHUNKS):
                jj = j - KCH
                strip = strips.tile([P, ROWS], cdt, name=f"rh_{j}", tag="strip")
                nc.sync.dma_start(strip[:], s2t[P * jj : P * (jj + 1), :])
                for nt in range(RCH):
                    nc.tensor.matmul(
                        accs3[nt][:],
                        mmcast(strip[:, P * nt : P * (nt + 1)]),
                        mmcast(t_sb[:, j, :]),
                        start=(idx == 0),
                        stop=(idx == len(HI_CHUNKS) - 1),
                    )
            for nt in range(RCH):
                # alternate engines so the 8 stash reads drain two at a time
                if nt % 2 == 0:
                    nc.vector.tensor_scalar(
                        stash[nt][:], accs3[nt][:], a128[:, 1:2], None,
                        mybir.AluOpType.mult,
                    )
                else:
                    nc.scalar.mul(stash[nt][:], accs3[nt][:], a128[:, 1:2])
            for idx, (j, g, c, u) in enumerate(LO_CHUNKS):
                strip = strips.tile([P, ROWS], cdt, name=f"rl_{j}", tag="strip")
                nc.sync.dma_start(strip[:], s0t[P * j : P * (j + 1), :])
                for nt in range(RCH):
                    nc.tensor.matmul(
                        accs3[nt][:],
                        mmcast(strip[:, P * nt : P * (nt + 1)]),
                        mmcast(t_sb[:, j, :]),
                        start=(idx == 0),
                        stop=(idx == len(LO_CHUNKS) - 1),
                    )
            for nt in range(RCH):
                lo = stage.tile([P, D], f32, name=f"elo_{nt}", tag="elo")
                # fused (acc * a0) max stash in one DVE pass
                nc.vector.scalar_tensor_tensor(
                    lo[:], accs3[nt][:], a128[:, 0:1], stash[nt][:],
                    mybir.AluOpType.mult, mybir.AluOpType.max,
                )
                nc.vector.tensor_tensor(
                    lo[:], lo[:], bias128[:], mybir.AluOpType.add
                )
                osb = stage.tile([P, D], f32, name=f"osb_{nt}", tag="osb")
                nc.scalar.activation(
                    osb[:], lo[:], mybir.ActivationFunctionType.Relu
                )
                row0 = P * nt
                nc.sync.dma_start(out[row0 : row0 + P, :], osb[:])

    nc.compile()
    return nc


def _get_nc(compute):
    if compute not in _CACHE:
        _CACHE[compute] = _build_nc(compute)
    return _CACHE[compute]


def _shard_inputs(x, weights, alpha, bias, s0, s1, s2, s3, compute):
    import ml_dtypes

    cnp = ml_dtypes.bfloat16 if compute == "bf16" else np.float32

    def prep(a):  # transpose + cast, C-contiguous
        return np.ascontiguousarray(a.T).astype(cnp, copy=False)

    alpha = np.ascontiguousarray(alpha, dtype=np.float32)
    bias = np.ascontiguousarray(bias, dtype=np.float32)
    w_p = np.ascontiguousarray(weights).astype(cnp, copy=False)  # natural: rhs is contract-major
    xt_full = prep(x)  # [D, N], replicated to every core
    in_maps = []
    for c in range(N_CORES):
        r0, r1 = ROWS * c, ROWS * (c + 1)
        # S = concat(s1, s3) rows; core c owns rows [r0, r1)
        if r1 <= K:
            s_rows = s1[r0:r1]
        elif r0 >= K:
            s_rows = s3[r0 - K : r1 - K]
        else:  # straddles the boundary (not the case for these shapes)
            s_rows = np.concatenate([s1[r0:], s3[: r1 - K]], axis=0)
        in_maps.append(
            {
                "xt": xt_full,
                "w": w_p,
                "alpha": alpha,
                "bias": bias,
                "st": prep(s_rows),
                "s0t": prep(s0[r0:r1]),
                "s2t": prep(s2[r0:r1]),
            }
        )
    return in_maps


def kernel(x, weights, alpha, bias, s0, s1, s2, s3, _trace=False):
    from concourse.bass_utils import run_bass_kernel_spmd

    compute = COMPUTE
    nc = _get_nc(compute)
    in_maps = _shard_inputs(
        np.asarray(x), np.asarray(weights), np.asarray(alpha), np.asarray(bias),
        np.asarray(s0), np.asarray(s1), np.asarray(s2), np.asarray(s3), compute,
    )
    kwargs = {}
    if _trace:
        # warm-up execution: compile + collective init + allocator warm so the
        # traced run measures steady-state
        run_bass_kernel_spmd(nc, in_maps, core_ids=list(range(N_CORES)))
        kwargs = dict(trace=True, trace_cores=list(range(N_CORES)))
    r = run_bass_kernel_spmd(nc, in_maps, core_ids=list(range(N_CORES)), **kwargs)
    full = np.concatenate([res["out"] for res in r.results], axis=0)
    if _trace:
        return full, r
    return full



# revision 6
# speedup vs baseline: 1.2398x; 1.2398x over previous
"""SpGAT_Conv Trainium2 kernel: 8-core SPMD spectral GNN conv.

Math (reference):
    a = softmax(alpha)
    pre = x @ W                                   [N, D]
    out_low  = s0 @ (a0 * (s1 @ pre))             [N, D]
    out_high = s2 @ (a1 * (s3 @ pre))             [N, D]
    out = relu(max(out_low, out_high) + bias)

Re-association: t = S @ (x @ W) == (S @ x) @ W with S = concat(s1, s3).
Row-sharding t's rows across 8 cores makes the x@W work perfectly sharded
too (it rides on each core's own 1024 rows of u = S_c @ x) instead of being
replicated, cutting per-core PE work from 1280 to 1056 big matmuls:

    step 1: u_c^T = x^T S_c^T accumulated over n-chunks; stationary = x
            chunks (natural layout), moving = S_c^T strips.  Two sweeps of
            512 i-columns each (4 PSUM banks per sweep).
    step 2: t_c = u_c @ W via u^T slices stationary, W moving (32 matmuls);
            each sweep's 512 t rows are staged + AllGathered in two
            sub-collectives (4 total, order 0,1,3,2) while later compute
            runs.
    phase 3: out_c = relu(max(a0*s0_c@t1 + bias, a1*s2_c@t3 + bias)), high
             band then low band over ARRIVAL-ordered t chunks; PSUM is
             pre-seeded with bias/a so no separate bias add exists in the
             epilogue.

All big operands are host-cast to bf16 (full PE rate) with fp32 PSUM
accumulation; s-matrices are host-transposed so the contraction dim lands
on SBUF partitions with contiguous DMAs.  x needs no transpose in this
formulation.

A dependency-free tiny AllGather is issued at kernel start to absorb the
first-collective init cost.
"""

import os

import numpy as np

N_CORES = 8
N = 8192
K = 2048
NK = N - K          # 6144
D = 512
ROWS = N // N_CORES  # 1024 rows per core
P = 128
RCH = ROWS // P      # 8  (row chunks per core / output strips)
NCH = N // P         # 64 (contraction chunks over full N)
KCH = K // P         # 16 (low-band chunks; high band = NCH - KCH = 48)
DCH = D // P         # 4  (depth chunks)
NSUB = 4             # sub-AllGathers for t
SUBR = ROWS // NSUB  # 256 rows per rank per sub-AG

DEBUG = os.environ.get("SPGAT_DEBUG", "0") == "1"

_CACHE = {}

# t-chunk arrival order: sub-AG g delivers, for every rank c, t rows
# [1024c + 256g, 1024c + 256(g+1)) = global chunks 8c + 2g + {0,1}.
# Gathers are triggered in order 0, 1, 3, 2 (sweep B stages its upper rows
# first), so consume in that order too.
GORDER = [0, 1, 3, 2]
ARRIVAL = [
    (8 * c + 2 * g + u, g, c, u)
    for g in GORDER
    for c in range(N_CORES)
    for u in range(2)
]


def _build_nc():
    import concourse.mybir as mybir
    import concourse.tile as tile
    from concourse import bacc

    f32 = mybir.dt.float32
    bf16 = mybir.dt.bfloat16
    cdt = bf16

    nc = bacc.Bacc(
        "TRN2", target_bir_lowering=False, debug=False, num_devices=N_CORES
    )

    x = nc.dram_tensor("x", [N, D], cdt, kind="ExternalInput").ap()
    w = nc.dram_tensor("w", [D, D], cdt, kind="ExternalInput").ap()
    alpha = nc.dram_tensor("alpha", [2], f32, kind="ExternalInput").ap()
    bias = nc.dram_tensor("bias", [D], f32, kind="ExternalInput").ap()
    st = nc.dram_tensor("st", [N, ROWS], cdt, kind="ExternalInput").ap()
    s0t = nc.dram_tensor("s0t", [K, ROWS], cdt, kind="ExternalInput").ap()
    s2t = nc.dram_tensor("s2t", [NK, ROWS], cdt, kind="ExternalInput").ap()
    out = nc.dram_tensor("out", [ROWS, D], f32, kind="ExternalOutput").ap()
    if DEBUG:
        t_dump = nc.dram_tensor("t_dump", [N, D], cdt, kind="ExternalOutput").ap()

    groups = [list(range(N_CORES))]

    with tile.TileContext(nc) as tc:
        with (
            tc.tile_pool(name="const", bufs=1) as const,
            tc.tile_pool(name="bigA", bufs=1) as bigA,
            tc.tile_pool(name="bigB", bufs=1) as bigB,
            tc.tile_pool(name="strips1", bufs=6) as strips1,
            tc.tile_pool(name="strips3", bufs=7) as strips3,
            tc.tile_pool(name="stage", bufs=4) as stage,
            tc.tile_pool(name="epi", bufs=2) as epi,
            tc.tile_pool(name="stash", bufs=1) as stashp,
            tc.tile_pool(name="ps", bufs=8, space="PSUM") as ps,
            tc.tile_pool(name="dram", bufs=1, space="DRAM") as dram,
        ):
            # ---- collective warm-up: absorb first-collective init.  No
            # input deps so the trigger is the very first gpsimd instruction.
            warm_in = dram.tile([8, 8], f32, name="warm_in")
            warm_out = dram.tile([64, 8], f32, name="warm_out", addr_space="Shared")
            nc.gpsimd.collective_compute(
                "AllGather",
                mybir.AluOpType.bypass,
                replica_groups=groups,
                ins=[warm_in.opt()],
                outs=[warm_out.opt()],
            )

            # ---- setup: softmax(alpha); broadcast a, bias/a0, bias/a1 to
            # 128 partitions via tiny ones-matmuls
            asb = const.tile([1, 2], f32, name="asb")
            nc.sync.dma_start(asb[:], alpha[None, :])
            bsb = const.tile([1, D], f32, name="bsb")
            nc.sync.dma_start(bsb[:], bias[None, :])

            amax = const.tile([1, 1], f32, name="amax")
            nc.vector.tensor_tensor(
                amax[:], asb[:, 0:1], asb[:, 1:2], mybir.AluOpType.max
            )
            ash = const.tile([1, 2], f32, name="ash")
            nc.vector.tensor_scalar(
                ash[:], asb[:], amax[:, 0:1], None, mybir.AluOpType.subtract
            )
            aexp = const.tile([1, 2], f32, name="aexp")
            nc.scalar.activation(aexp[:], ash[:], mybir.ActivationFunctionType.Exp)
            asum = const.tile([1, 1], f32, name="asum")
            nc.vector.tensor_tensor(
                asum[:], aexp[:, 0:1], aexp[:, 1:2], mybir.AluOpType.add
            )
            arec = const.tile([1, 1], f32, name="arec")
            nc.vector.reciprocal(arec[:], asum[:])
            afin = const.tile([1, 2], f32, name="afin")
            nc.vector.tensor_scalar(
                afin[:], aexp[:], arec[:, 0:1], None, mybir.AluOpType.mult
            )
            # 1/a0, 1/a1 then bias/a0, bias/a1 (PSUM seed values)
            ainv = const.tile([1, 2], f32, name="ainv")
            nc.vector.reciprocal(ainv[:], afin[:])
            srow0 = const.tile([1, D], f32, name="srow0", tag="seedrow")
            nc.vector.tensor_scalar(
                srow0[:], bsb[:], ainv[:, 0:1], None, mybir.AluOpType.mult
            )
            srow1 = const.tile([1, D], f32, name="srow1", tag="seedrow1")
            nc.vector.tensor_scalar(
                srow1[:], bsb[:], ainv[:, 1:2], None, mybir.AluOpType.mult
            )

            ones = const.tile([1, P], f32, name="ones")
            nc.vector.memset(ones[:], 1.0)
            ps_a = ps.tile([P, 2], f32, name="ps_a", tag="acc")
            nc.tensor.matmul(ps_a[:], ones[:], afin[:], start=True, stop=True)
            a128 = const.tile([P, 2], f32, name="a128")
            nc.vector.tensor_copy(a128[:], ps_a[:])
            seeds = const.tile([P, 2 * D], f32, name="seeds")
            ps_b0 = ps.tile([P, D], f32, name="ps_b0", tag="acc")
            nc.tensor.matmul(ps_b0[:], ones[:], srow0[:], start=True, stop=True)
            nc.vector.tensor_copy(seeds[:, 0:D], ps_b0[:])
            ps_b1 = ps.tile([P, D], f32, name="ps_b1", tag="acc")
            nc.tensor.matmul(ps_b1[:], ones[:], srow1[:], start=True, stop=True)
            nc.vector.tensor_copy(seeds[:, D : 2 * D], ps_b1[:])
            seed_lo = seeds[:, 0:D]
            seed_hi = seeds[:, D : 2 * D]

            # ---- weights + x resident in SBUF
            w_sb = const.tile([P, DCH, D], cdt, name="w_sb")
            nc.sync.dma_start(w_sb[:], w.rearrange("(c p) d -> p c d", p=P))
            x_sb = bigA.tile([P, NCH, D], cdt, name="x_sb", tag="bigA")
            XB = 16  # chunked loads so step-1 can start early
            x_v = x.rearrange("(b c p) d -> b p c d", p=P, b=XB)
            xw = NCH // XB
            for b in range(XB):
                if b == 0:  # split so the first step-1 matmul starts sooner
                    nc.sync.dma_start(x_sb[:, 0:1, :], x_v[0][:, 0:1, :])
                    nc.sync.dma_start(x_sb[:, 1:2, :], x_v[0][:, 1:2, :])
                    nc.sync.dma_start(x_sb[:, 2:4, :], x_v[0][:, 2:4, :])
                else:
                    nc.sync.dma_start(x_sb[:, xw * b : xw * (b + 1), :], x_v[b])

            # ---- steps 1+2: u_c^T = x^T S_c^T over n-chunks, then
            # t_c = u_c @ W; two sweeps over i-columns (512 each)
            t_in = dram.tile([ROWS, D], cdt, name="t_in")
            t_outs = [
                dram.tile([SUBR * N_CORES, D], cdt, name=f"t_out{g}",
                          addr_space="Shared")
                for g in range(NSUB)
            ]

            def t_subag(g):
                nc.gpsimd.collective_compute(
                    "AllGather",
                    mybir.AluOpType.bypass,
                    replica_groups=groups,
                    ins=[t_in[SUBR * g : SUBR * (g + 1), :].opt()],
                    outs=[t_outs[g].opt()],
                )

            ut_sb = [
                const.tile([P, DCH, D], cdt, name=f"ut{sw}") for sw in range(2)
            ]

            def sweep1(sw, accs, j0, j1):
                """step-1 matmuls for n-chunks [j0, j1) of sweep sw."""
                col0 = D * sw
                for j in range(j0, j1):
                    strip = strips1.tile([P, D], cdt, name=f"s{sw}_{j}",
                                         tag="strip")
                    nc.sync.dma_start(
                        strip[:], st[P * j : P * (j + 1), col0 : col0 + D]
                    )
                    for dc in range(DCH):
                        nc.tensor.matmul(
                            accs[dc][:],
                            x_sb[:, j, P * dc : P * (dc + 1)],
                            strip[:],
                            start=(j == 0),
                            stop=(j == NCH - 1),
                        )

            def drain_u(sw, accs):
                for dc in range(DCH):
                    if dc % 2 == 0:
                        nc.vector.tensor_copy(ut_sb[sw][:, dc, :], accs[dc][:])
                    else:
                        nc.scalar.copy(ut_sb[sw][:, dc, :], accs[dc][:])

            def step2(sw):
                """t rows [512sw, 512sw+512): 4 i-blocks of 128 rows; stage
                + trigger this sweep's two sub-AGs (B stages upper first)."""
                ib_order = [0, 1, 2, 3] if sw == 0 else [2, 3, 0, 1]
                ags = [(1, 2 * sw), (3, 2 * sw + 1)] if sw == 0 else \
                      [(1, 3), (3, 2)]
                ag_map = dict(ags)
                for i, ib in enumerate(ib_order):
                    tp = ps.tile([P, D], f32, name=f"t_ps_{sw}_{ib}", tag="acc")
                    for dc in range(DCH):
                        nc.tensor.matmul(
                            tp[:],
                            ut_sb[sw][:, dc, P * ib : P * (ib + 1)],
                            w_sb[:, dc, :],
                            start=(dc == 0),
                            stop=(dc == DCH - 1),
                        )
                    tst = stage.tile([P, D], cdt, name=f"t_st_{sw}_{ib}",
                                     tag="st")
                    if i % 2 == 0:
                        nc.vector.tensor_copy(tst[:], tp[:])
                    else:
                        nc.scalar.copy(tst[:], tp[:])
                    row0 = D * sw + P * ib
                    nc.sync.dma_start(t_in[row0 : row0 + P, :], tst[:])
                    if i in ag_map:
                        t_subag(ag_map[i])

            accsA = [ps.tile([P, D], f32, name=f"uA_{dc}", tag="acc")
                     for dc in range(DCH)]
            accsB = [ps.tile([P, D], f32, name=f"uB_{dc}", tag="acc")
                     for dc in range(DCH)]

            sweep1(0, accsA, 0, NCH)
            drain_u(0, accsA)
            # head of sweep B hides sweep A's drain latency before step2A
            sweep1(1, accsB, 0, 8)
            step2(0)
            sweep1(1, accsB, 8, NCH)
            drain_u(1, accsB)
            step2(1)

            # ---- phase 3: out_c = relu(max(a0*s0_c@t1 + b, a1*s2_c@t3 + b))
            t_sb = bigB.tile([P, NCH, D], cdt, name="t_sb", tag="bigB")
            for j, g, c, u in ARRIVAL:
                nc.sync.dma_start(
                    t_sb[:, j, :],
                    t_outs[g][SUBR * c + P * u : SUBR * c + P * (u + 1), :],
                )
            if DEBUG:
                for j, g, c, u in ARRIVAL:
                    nc.sync.dma_start(
                        t_dump[P * j : P * (j + 1), :], t_sb[:, j, :]
                    )

            # high band first across all 8 out strips (its matmuls cover the
            # t-gather pipeline), stash a1*high+bias to SBUF, then the low
            # band reuses the same 8 PSUM banks.  Accumulators are pre-seeded
            # with bias/a so the scale folds the bias in for free.
            HI_CHUNKS = [e for e in ARRIVAL if e[0] >= KCH]
            LO_CHUNKS = [e for e in ARRIVAL if e[0] < KCH]
            accs3 = [
                ps.tile([P, D], f32, name=f"acc3_{nt}", tag="acc")
                for nt in range(RCH)
            ]
            stash = [
                stashp.tile([P, D], f32, name=f"hst_{nt}", tag=f"hst{nt}")
                for nt in range(RCH)
            ]
            for nt in range(RCH):
                if nt % 2 == 0:
                    nc.vector.tensor_copy(accs3[nt][:], seed_hi)
                else:
                    nc.scalar.copy(accs3[nt][:], seed_hi)
            for idx, (j, g, c, u) in enumerate(HI_CHUNKS):
                jj = j - KCH
                strip = strips3.tile([P, ROWS], cdt, name=f"rh_{j}", tag="strip3")
                nc.sync.dma_start(strip[:], s2t[P * jj : P * (jj + 1), :])
                for nt in range(RCH):
                    nc.tensor.matmul(
                        accs3[nt][:],
                        strip[:, P * nt : P * (nt + 1)],
                        t_sb[:, j, :],
                        start=False,
                        stop=(idx == len(HI_CHUNKS) - 1),
                    )
            for nt in range(RCH):
                # stash = a1*acc = a1*hi + bias; alternate engines
                if nt % 2 == 0:
                    nc.vector.tensor_scalar(
                        stash[nt][:], accs3[nt][:], a128[:, 1:2], None,
                        mybir.AluOpType.mult,
                    )
                else:
                    nc.scalar.mul(stash[nt][:], accs3[nt][:], a128[:, 1:2])
                # re-seed for the low band right behind the stash read
                if nt % 2 == 0:
                    nc.scalar.copy(accs3[nt][:], seed_lo)
                else:
                    nc.vector.tensor_copy(accs3[nt][:], seed_lo)
            for idx, (j, g, c, u) in enumerate(LO_CHUNKS):
                strip = strips3.tile([P, ROWS], cdt, name=f"rl_{j}", tag="strip3")
                nc.sync.dma_start(strip[:], s0t[P * j : P * (j + 1), :])
                for nt in range(RCH):
                    nc.tensor.matmul(
                        accs3[nt][:],
                        strip[:, P * nt : P * (nt + 1)],
                        t_sb[:, j, :],
                        start=False,
                        stop=(idx == len(LO_CHUNKS) - 1),
                    )
            for nt in range(RCH):
                lo = epi.tile([P, D], f32, name=f"elo_{nt}", tag="elo")
                # fused (acc * a0) max stash in one DVE pass; bias already in
                nc.vector.scalar_tensor_tensor(
                    lo[:], accs3[nt][:], a128[:, 0:1], stash[nt][:],
                    mybir.AluOpType.mult, mybir.AluOpType.max,
                )
                osb = epi.tile([P, D], f32, name=f"osb_{nt}", tag="osb")
                nc.scalar.activation(
                    osb[:], lo[:], mybir.ActivationFunctionType.Relu
                )
                row0 = P * nt
                nc.sync.dma_start(out[row0 : row0 + P, :], osb[:])

    nc.compile()
    return nc


def _get_nc():
    if "nc" not in _CACHE:
        _CACHE["nc"] = _build_nc()
    return _CACHE["nc"]


def _shard_inputs(x, weights, alpha, bias, s0, s1, s2, s3):
    import ml_dtypes

    cnp = ml_dtypes.bfloat16

    def prep(a):  # transpose + cast, C-contiguous
        return np.ascontiguousarray(a.T).astype(cnp, copy=False)

    alpha = np.ascontiguousarray(alpha, dtype=np.float32)
    bias = np.ascontiguousarray(bias, dtype=np.float32)
    w_p = np.ascontiguousarray(weights).astype(cnp, copy=False)
    x_p = np.ascontiguousarray(x).astype(cnp, copy=False)  # natural layout
    in_maps = []
    for c in range(N_CORES):
        r0, r1 = ROWS * c, ROWS * (c + 1)
        # S = concat(s1, s3) rows; core c owns rows [r0, r1)
        if r1 <= K:
            s_rows = s1[r0:r1]
        elif r0 >= K:
            s_rows = s3[r0 - K : r1 - K]
        else:  # straddles the boundary (not the case for these shapes)
            s_rows = np.concatenate([s1[r0:], s3[: r1 - K]], axis=0)
        in_maps.append(
            {
                "x": x_p,
                "w": w_p,
                "alpha": alpha,
                "bias": bias,
                "st": prep(s_rows),
                "s0t": prep(s0[r0:r1]),
                "s2t": prep(s2[r0:r1]),
            }
        )
    return in_maps


def kernel(x, weights, alpha, bias, s0, s1, s2, s3, _trace=False):
    from concourse.bass_utils import run_bass_kernel_spmd

    nc = _get_nc()
    in_maps = _shard_inputs(
        np.asarray(x), np.asarray(weights), np.asarray(alpha), np.asarray(bias),
        np.asarray(s0), np.asarray(s1), np.asarray(s2), np.asarray(s3),
    )
    kwargs = {}
    if _trace:
        # warm-up execution: compile + collective init + allocator warm so the
        # traced run measures steady-state
        run_bass_kernel_spmd(nc, in_maps, core_ids=list(range(N_CORES)))
        kwargs = dict(trace=True, trace_cores=list(range(N_CORES)))
    r = run_bass_kernel_spmd(nc, in_maps, core_ids=list(range(N_CORES)), **kwargs)
    full = np.concatenate([res["out"] for res in r.results], axis=0)
    if _trace:
        return full, r
    return full


# revision 9
# speedup vs baseline: 1.3146x; 1.0603x over previous
"""SpGAT_Conv Trainium2 kernel: 8-core SPMD spectral GNN conv.

Math (reference):
    a = softmax(alpha)
    pre = x @ W                                   [N, D]
    out_low  = s0 @ (a0 * (s1 @ pre))             [N, D]
    out_high = s2 @ (a1 * (s3 @ pre))             [N, D]
    out = relu(max(out_low, out_high) + bias)

Re-association: t = S @ (x @ W) == (S @ x) @ W with S = concat(s1, s3).
Row-sharding t's rows across 8 cores makes the x@W work perfectly sharded
too (it rides on each core's own 1024 rows of u = S_c @ x) instead of being
replicated, cutting per-core PE work from 1280 to 1056 big matmuls:

    step 1: u_c^T = x^T S_c^T accumulated over n-chunks; stationary = x
            chunks (natural layout), moving = S_c^T strips.  Two sweeps of
            512 i-columns each (4 PSUM banks per sweep).
    step 2: t_c = u_c @ W via u^T slices stationary, W moving (32 matmuls);
            each sweep's 512 t rows are staged + AllGathered in two
            sub-collectives (4 total, order 0,1,3,2) while later compute
            runs.
    phase 3: out_c = relu(max(a0*s0_c@t1 + bias, a1*s2_c@t3 + bias)), high
             band then low band over ARRIVAL-ordered t chunks; PSUM is
             pre-seeded with bias/a so no separate bias add exists in the
             epilogue.

DMA queue discipline (DMA_DIRECT2D executes synchronously on the issuing
engine's queue, so a queue is a serial resource and a blocked DMA blocks
everything behind it):
  - Sync queue: only the matmul-pacing strip loads (st/s0t/s2t), t_in
    stores and out stores — nothing on it ever waits on a collective.
  - Activation queue: bulk x/w loads (head) and the AllGather-gated t_sb
    loads (tail) plus the relu epilogue.
  - Vector engine owns every PSUM drain/stage/seed/stash copy, emitted in
    PSUM-buffer-availability order.

All big operands are host-cast to bf16 (full PE rate) with fp32 PSUM
accumulation; s-matrices are host-transposed so the contraction dim lands
on SBUF partitions with contiguous DMAs.  x needs no transpose in this
formulation.

A dependency-free tiny AllGather is issued at kernel start to absorb the
first-collective init cost.
"""

import os

import numpy as np

N_CORES = 8
N = 8192
K = 2048
NK = N - K          # 6144
D = 512
ROWS = N // N_CORES  # 1024 rows per core
P = 128
RCH = ROWS // P      # 8  (row chunks per core / output strips)
NCH = N // P         # 64 (contraction chunks over full N)
KCH = K // P         # 16 (low-band chunks; high band = NCH - KCH = 48)
DCH = D // P         # 4  (depth chunks)
NSUB = 4             # sub-AllGathers for t
SUBR = ROWS // NSUB  # 256 rows per rank per sub-AG

DEBUG = os.environ.get("SPGAT_DEBUG", "0") == "1"

_CACHE = {}

# t-chunk arrival order: sub-AG g delivers, for every rank c, t rows
# [1024c + 256g, 1024c + 256(g+1)) = global chunks 8c + 2g + {0,1}.
# Gathers are triggered in order 0, 1, 3, 2 (sweep B stages its upper rows
# first), so consume in that order too.
GORDER = [0, 1, 3, 2]
ARRIVAL = [
    (8 * c + 2 * g + u, g, c, u)
    for g in GORDER
    for c in range(N_CORES)
    for u in range(2)
]


def _build_nc():
    import concourse.mybir as mybir
    import concourse.tile as tile
    from concourse import bacc

    f32 = mybir.dt.float32
    bf16 = mybir.dt.bfloat16
    cdt = bf16

    nc = bacc.Bacc(
        "TRN2", target_bir_lowering=False, debug=False, num_devices=N_CORES
    )

    x = nc.dram_tensor("x", [N, D], cdt, kind="ExternalInput").ap()
    w = nc.dram_tensor("w", [D, D], cdt, kind="ExternalInput").ap()
    alpha = nc.dram_tensor("alpha", [2], f32, kind="ExternalInput").ap()
    bias = nc.dram_tensor("bias", [D], f32, kind="ExternalInput").ap()
    st = nc.dram_tensor("st", [N, ROWS], cdt, kind="ExternalInput").ap()
    s0t = nc.dram_tensor("s0t", [K, ROWS], cdt, kind="ExternalInput").ap()
    s2t = nc.dram_tensor("s2t", [NK, ROWS], cdt, kind="ExternalInput").ap()
    out = nc.dram_tensor("out", [ROWS, D], f32, kind="ExternalOutput").ap()
    if DEBUG:
        t_dump = nc.dram_tensor("t_dump", [N, D], cdt, kind="ExternalOutput").ap()

    groups = [list(range(N_CORES))]

    with tile.TileContext(nc) as tc:
        with (
            tc.tile_pool(name="const", bufs=1) as const,
            tc.tile_pool(name="bigA", bufs=1) as bigA,
            tc.tile_pool(name="bigB", bufs=1) as bigB,
            tc.tile_pool(name="strips1", bufs=8) as strips1,
            tc.tile_pool(name="strips3", bufs=7) as strips3,
            tc.tile_pool(name="stage", bufs=4) as stage,
            tc.tile_pool(name="epi", bufs=2) as epi,
            tc.tile_pool(name="stash", bufs=1) as stashp,
            tc.tile_pool(name="ps", bufs=8, space="PSUM") as ps,
            tc.tile_pool(name="dram", bufs=1, space="DRAM") as dram,
        ):
            # ---- collective warm-up: absorb first-collective init.  No
            # input deps so the trigger is the very first gpsimd instruction.
            warm_in = dram.tile([8, 8], f32, name="warm_in")
            warm_out = dram.tile([64, 8], f32, name="warm_out", addr_space="Shared")
            nc.gpsimd.collective_compute(
                "AllGather",
                mybir.AluOpType.bypass,
                replica_groups=groups,
                ins=[warm_in.opt()],
                outs=[warm_out.opt()],
            )

            # ---- PSUM allocation order fixes the 8-bank rotation; all
            # tiles are one 2KB bank.
            ps_a = ps.tile([P, 2], f32, name="ps_a", tag="acc")
            ps_b0 = ps.tile([P, D], f32, name="ps_b0", tag="acc")
            ps_b1 = ps.tile([P, D], f32, name="ps_b1", tag="acc")
            accsA = [ps.tile([P, D], f32, name=f"uA_{dc}", tag="acc")
                     for dc in range(DCH)]
            accsB = [ps.tile([P, D], f32, name=f"uB_{dc}", tag="acc")
                     for dc in range(DCH)]

            # ---- input DMAs: alpha/bias tiny on sync; x/w bulk on the
            # scalar (Activation) HWDGE queue so the sync queue only ever
            # carries matmul-pacing strip loads.
            asb = const.tile([1, 2], f32, name="asb")
            nc.sync.dma_start(asb[:], alpha[None, :])
            bsb = const.tile([1, D], f32, name="bsb")
            nc.sync.dma_start(bsb[:], bias[None, :])

            x_sb = bigA.tile([P, NCH, D], cdt, name="x_sb", tag="bigA")
            XB = 16
            x_v = x.rearrange("(b c p) d -> b p c d", p=P, b=XB)
            xw = NCH // XB
            for b in range(XB):
                if b == 0:  # split so the first step-1 matmul starts sooner
                    nc.scalar.dma_start(x_sb[:, 0:1, :], x_v[0][:, 0:1, :])
                    nc.scalar.dma_start(x_sb[:, 1:2, :], x_v[0][:, 1:2, :])
                    nc.scalar.dma_start(x_sb[:, 2:4, :], x_v[0][:, 2:4, :])
                else:
                    nc.scalar.dma_start(x_sb[:, xw * b : xw * (b + 1), :], x_v[b])
            w_sb = const.tile([P, DCH, D], cdt, name="w_sb")
            nc.scalar.dma_start(w_sb[:], w.rearrange("(c p) d -> p c d", p=P))

            # ---- steps 1+2 infrastructure
            t_in = dram.tile([ROWS, D], cdt, name="t_in")
            t_outs = [
                dram.tile([SUBR * N_CORES, D], cdt, name=f"t_out{g}",
                          addr_space="Shared")
                for g in range(NSUB)
            ]

            def t_subag(g):
                nc.gpsimd.collective_compute(
                    "AllGather",
                    mybir.AluOpType.bypass,
                    replica_groups=groups,
                    ins=[t_in[SUBR * g : SUBR * (g + 1), :].opt()],
                    outs=[t_outs[g].opt()],
                )

            ut_sb = [
                const.tile([P, DCH, D], cdt, name=f"ut{sw}") for sw in range(2)
            ]

            def sweep1(sw, accs, j0, j1):
                """step-1 matmuls for n-chunks [j0, j1) of sweep sw."""
                col0 = D * sw
                for j in range(j0, j1):
                    strip = strips1.tile([P, D], cdt, name=f"s{sw}_{j}",
                                         tag="strip")
                    nc.sync.dma_start(
                        strip[:], st[P * j : P * (j + 1), col0 : col0 + D]
                    )
                    for dc in range(DCH):
                        nc.tensor.matmul(
                            accs[dc][:],
                            x_sb[:, j, P * dc : P * (dc + 1)],
                            strip[:],
                            start=(j == 0),
                            stop=(j == NCH - 1),
                        )

            def drain_u(sw, accs):
                for dc in range(DCH):
                    nc.vector.tensor_copy(ut_sb[sw][:, dc, :], accs[dc][:])

            def step2(sw, tps=None):
                """t rows [512sw, 512sw+512): 4 i-blocks of 128 rows; stage
                + trigger this sweep's two sub-AGs (B stages upper first)."""
                ib_order = [0, 1, 2, 3] if sw == 0 else [2, 3, 0, 1]
                ag_map = {1: 0, 3: 1} if sw == 0 else {1: 3, 3: 2}
                for i, ib in enumerate(ib_order):
                    if tps is None:
                        tp = ps.tile([P, D], f32, name=f"t_ps_{sw}_{ib}",
                                     tag="acc")
                    else:
                        tp = tps[i]
                    for dc in range(DCH):
                        nc.tensor.matmul(
                            tp[:],
                            ut_sb[sw][:, dc, P * ib : P * (ib + 1)],
                            w_sb[:, dc, :],
                            start=(dc == 0),
                            stop=(dc == DCH - 1),
                        )
                    tst = stage.tile([P, D], cdt, name=f"t_st_{sw}_{ib}",
                                     tag="st")
                    nc.vector.tensor_copy(tst[:], tp[:])
                    row0 = D * sw + P * ib
                    nc.sync.dma_start(t_in[row0 : row0 + P, :], tst[:])
                    if i in ag_map:
                        t_subag(ag_map[i])

            # ---- PE stream head: first sweep chunks go before the softmax
            # setup matmuls so the PE starts as soon as strip 0 lands.
            sweep1(0, accsA, 0, 4)

            # softmax(alpha) chain (vector) + broadcasts via tiny matmuls
            amax = const.tile([1, 1], f32, name="amax")
            nc.vector.tensor_tensor(
                amax[:], asb[:, 0:1], asb[:, 1:2], mybir.AluOpType.max
            )
            ash = const.tile([1, 2], f32, name="ash")
            nc.vector.tensor_scalar(
                ash[:], asb[:], amax[:, 0:1], None, mybir.AluOpType.subtract
            )
            aexp = const.tile([1, 2], f32, name="aexp")
            nc.scalar.activation(aexp[:], ash[:], mybir.ActivationFunctionType.Exp)
            asum = const.tile([1, 1], f32, name="asum")
            nc.vector.tensor_tensor(
                asum[:], aexp[:, 0:1], aexp[:, 1:2], mybir.AluOpType.add
            )
            arec = const.tile([1, 1], f32, name="arec")
            nc.vector.reciprocal(arec[:], asum[:])
            afin = const.tile([1, 2], f32, name="afin")
            nc.vector.tensor_scalar(
                afin[:], aexp[:], arec[:, 0:1], None, mybir.AluOpType.mult
            )
            ainv = const.tile([1, 2], f32, name="ainv")
            nc.vector.reciprocal(ainv[:], afin[:])
            srow0 = const.tile([1, D], f32, name="srow0")
            nc.vector.tensor_scalar(
                srow0[:], bsb[:], ainv[:, 0:1], None, mybir.AluOpType.mult
            )
            srow1 = const.tile([1, D], f32, name="srow1")
            nc.vector.tensor_scalar(
                srow1[:], bsb[:], ainv[:, 1:2], None, mybir.AluOpType.mult
            )
            ones = const.tile([1, P], f32, name="ones")
            nc.vector.memset(ones[:], 1.0)
            nc.tensor.matmul(ps_a[:], ones[:], afin[:], start=True, stop=True)
            a128 = const.tile([P, 2], f32, name="a128")
            nc.vector.tensor_copy(a128[:], ps_a[:])
            seeds = const.tile([P, 2 * D], f32, name="seeds")
            nc.tensor.matmul(ps_b0[:], ones[:], srow0[:], start=True, stop=True)
            nc.vector.tensor_copy(seeds[:, 0:D], ps_b0[:])
            nc.tensor.matmul(ps_b1[:], ones[:], srow1[:], start=True, stop=True)
            nc.vector.tensor_copy(seeds[:, D : 2 * D], ps_b1[:])
            seed_lo = seeds[:, 0:D]
            seed_hi = seeds[:, D : 2 * D]

            # ---- rest of sweep A, then pipelined B/step2
            sweep1(0, accsA, 4, NCH)
            drain_u(0, accsA)
            # head of sweep B hides sweep A's drain latency before step2A
            sweep1(1, accsB, 0, 8)
            step2(0)
            sweep1(1, accsB, 8, NCH)
            drain_u(1, accsB)

            # PSUM rotation: allocate step2B's tiles, then phase-3
            # accumulators — accs3[0..3] land on step2A's banks (free
            # early), accs3[4..7] on step2B's.  Seed 0..3 before step2B's
            # instructions so only 4..7 wait on its staging.
            tps1 = [ps.tile([P, D], f32, name=f"t_ps_1_{i}", tag="acc")
                    for i in range(4)]
            accs3 = [
                ps.tile([P, D], f32, name=f"acc3_{nt}", tag="acc")
                for nt in range(RCH)
            ]
            for nt in range(4):
                nc.vector.tensor_copy(accs3[nt][:], seed_hi)
            step2(1, tps1)
            for nt in range(4, RCH):
                nc.vector.tensor_copy(accs3[nt][:], seed_hi)

            # ---- phase 3: t gather consumption + band matmuls
            t_sb = bigB.tile([P, NCH, D], cdt, name="t_sb", tag="bigB")
            for j, g, c, u in ARRIVAL:
                nc.scalar.dma_start(
                    t_sb[:, j, :],
                    t_outs[g][SUBR * c + P * u : SUBR * c + P * (u + 1), :],
                )
            if DEBUG:
                for j, g, c, u in ARRIVAL:
                    nc.sync.dma_start(
                        t_dump[P * j : P * (j + 1), :], t_sb[:, j, :]
                    )

            HI_CHUNKS = [e for e in ARRIVAL if e[0] >= KCH]
            LO_CHUNKS = [e for e in ARRIVAL if e[0] < KCH]
            stash = [
                stashp.tile([P, D], f32, name=f"hst_{nt}", tag=f"hst{nt}")
                for nt in range(RCH)
            ]
            for idx, (j, g, c, u) in enumerate(HI_CHUNKS):
                jj = j - KCH
                strip = strips3.tile([P, ROWS], cdt, name=f"rh_{j}", tag="strip3")
                nc.sync.dma_start(strip[:], s2t[P * jj : P * (jj + 1), :])
                for nt in range(RCH):
                    nc.tensor.matmul(
                        accs3[nt][:],
                        strip[:, P * nt : P * (nt + 1)],
                        t_sb[:, j, :],
                        start=False,
                        stop=(idx == len(HI_CHUNKS) - 1),
                    )
            for nt in range(RCH):
                # stash = a1*acc = a1*hi + bias, then re-seed for the low
                # band right behind the stash read
                nc.vector.tensor_scalar(
                    stash[nt][:], accs3[nt][:], a128[:, 1:2], None,
                    mybir.AluOpType.mult,
                )
                nc.vector.tensor_copy(accs3[nt][:], seed_lo)
            for idx, (j, g, c, u) in enumerate(LO_CHUNKS):
                strip = strips3.tile([P, ROWS], cdt, name=f"rl_{j}", tag="strip3")
                nc.sync.dma_start(strip[:], s0t[P * j : P * (j + 1), :])
                for nt in range(RCH):
                    nc.tensor.matmul(
                        accs3[nt][:],
                        strip[:, P * nt : P * (nt + 1)],
                        t_sb[:, j, :],
                        start=False,
                        stop=(idx == len(LO_CHUNKS) - 1),
                    )
            for nt in range(RCH):
                lo = epi.tile([P, D], f32, name=f"elo_{nt}", tag="elo")
                # fused (acc * a0) max stash in one DVE pass; bias already in
                nc.vector.scalar_tensor_tensor(
                    lo[:], accs3[nt][:], a128[:, 0:1], stash[nt][:],
                    mybir.AluOpType.mult, mybir.AluOpType.max,
                )
                osb = epi.tile([P, D], f32, name=f"osb_{nt}", tag="osb")
                nc.scalar.activation(
                    osb[:], lo[:], mybir.ActivationFunctionType.Relu
                )
                row0 = P * nt
                nc.sync.dma_start(out[row0 : row0 + P, :], osb[:])

    nc.compile()
    return nc


def _get_nc():
    if "nc" not in _CACHE:
        _CACHE["nc"] = _build_nc()
    return _CACHE["nc"]


def _shard_inputs(x, weights, alpha, bias, s0, s1, s2, s3):
    import ml_dtypes

    cnp = ml_dtypes.bfloat16

    def prep(a):  # transpose + cast, C-contiguous
        return np.ascontiguousarray(a.T).astype(cnp, copy=False)

    alpha = np.ascontiguousarray(alpha, dtype=np.float32)
    bias = np.ascontiguousarray(bias, dtype=np.float32)
    w_p = np.ascontiguousarray(weights).astype(cnp, copy=False)
    x_p = np.ascontiguousarray(x).astype(cnp, copy=False)  # natural layout
    in_maps = []
    for c in range(N_CORES):
        r0, r1 = ROWS * c, ROWS * (c + 1)
        # S = concat(s1, s3) rows; core c owns rows [r0, r1)
        if r1 <= K:
            s_rows = s1[r0:r1]
        elif r0 >= K:
            s_rows = s3[r0 - K : r1 - K]
        else:  # straddles the boundary (not the case for these shapes)
            s_rows = np.concatenate([s1[r0:], s3[: r1 - K]], axis=0)
        in_maps.append(
            {
                "x": x_p,
                "w": w_p,
                "alpha": alpha,
                "bias": bias,
                "st": prep(s_rows),
                "s0t": prep(s0[r0:r1]),
                "s2t": prep(s2[r0:r1]),
            }
        )
    return in_maps


def kernel(x, weights, alpha, bias, s0, s1, s2, s3, _trace=False):
    from concourse.bass_utils import run_bass_kernel_spmd

    nc = _get_nc()
    in_maps = _shard_inputs(
        np.asarray(x), np.asarray(weights), np.asarray(alpha), np.asarray(bias),
        np.asarray(s0), np.asarray(s1), np.asarray(s2), np.asarray(s3),
    )
    kwargs = {}
    if _trace:
        # warm-up execution: compile + collective init + allocator warm so the
        # traced run measures steady-state
        run_bass_kernel_spmd(nc, in_maps, core_ids=list(range(N_CORES)))
        kwargs = dict(trace=True, trace_cores=list(range(N_CORES)))
    r = run_bass_kernel_spmd(nc, in_maps, core_ids=list(range(N_CORES)), **kwargs)
    full = np.concatenate([res["out"] for res in r.results], axis=0)
    if _trace:
        return full, r
    return full
